# revision 1
# baseline (speedup 1.0000x reference)
"""GATNet (2-layer GAT, 50000 nodes / 800000 edges) on 8 Trainium2 cores.

Strategy: dst-sharding, edges bucketed per 128-dst block on host.

Layer 1 is gather-free: the host ships per-edge source features x_eT
(pure data movement / indexing, bf16) plus one-hot ST selector streams;
the device computes per-tile records h|al_src = x_tile @ W1e on PE,
al_dst via ST-expansion matmuls, attention weighting on DVE, and the
per-dst segment sums (messages + softmax denominators) as S-matmul
accumulation into PSUM. S is built on-chip from a tiny j-stream via
is_equal against an iota table.

Layer 2 records [z | al_src2] (device data) are AllGathered as a
[50000, 128]-bf16 table (256B rows) and fetched per edge with the Q7
dma_gather (1024-idx calls); attention and aggregation mirror layer 1.
"""

import sys
import numpy as np

sys.path.insert(0, "/opt/trn_rl_repo")

NCORES = 8
BLK = 128
TILE = 128
LO_LIM = 32768
HEADS, HID, OUT_CH = 8, 32, 16
F1 = HEADS * HID            # 256
R1_W = F1 + HEADS           # 264 (h | al_src)
REC_W = 32                  # layer-2 record row (18 used, 64B)
REC_W2 = OUT_CH + 1         # cols consumed per record in E2
NEG_SLOPE = 0.2
DEN_EPS = 1e-30
PAD_J = 200.0


class _P:
    pass


# ---------------------------------------------------------------- planning

def _tiling(s_k, d_k, nblk, split_lohi, n_nodes, lo_lim):
    """Per-core slot layout: per block, tiles of <=128 edges; optionally
    split into lo/hi src-kind tiles (for int16 Q7 gathers). Returns
    per-core counts array [nblk, (1 or 2)]."""
    if split_lohi:
        kind = (s_k >= lo_lim).astype(np.int64)
    else:
        kind = np.zeros(len(s_k), np.int64)
    key = (d_k // BLK) * 4 * n_nodes + kind * 2 * n_nodes + s_k
    order = np.argsort(key, kind="stable")
    s_k, d_k, kind = s_k[order], d_k[order], kind[order]
    nk = 2 if split_lohi else 1
    cnt = np.zeros((nblk, nk), np.int64)
    np.add.at(cnt, (d_k // BLK, kind), 1)
    return s_k, d_k, kind, cnt


def _plan(edge_index, n_nodes):
    ndst = n_nodes // NCORES
    nblk = (ndst + BLK - 1) // BLK
    npad = nblk * BLK
    src = np.concatenate([edge_index[0], np.arange(n_nodes)]).astype(np.int64)
    dst = np.concatenate([edge_index[1], np.arange(n_nodes)]).astype(np.int64)
    owner = dst // ndst

    pl = _P()
    pl.ndst, pl.nblk, pl.npad, pl.n_nodes = ndst, nblk, npad, n_nodes
    pl.lo_lim = min(LO_LIM, n_nodes)
    per_core = []
    cnt_all = np.zeros((NCORES, nblk, 1), np.int64)
    for k in range(NCORES):
        m = owner == k
        s_k, d_k, kind, cnt = _tiling(src[m], dst[m] - k * ndst, nblk,
                                      False, n_nodes, pl.lo_lim)
        per_core.append((s_k, d_k, kind))
        cnt_all[k] = cnt
    # per-block tile counts (max over cores) - shared instr stream
    T_b = np.maximum(-(-cnt_all[:, :, 0].max(axis=0) // TILE), 1)
    pl.T_b = T_b
    pl.off_b = np.concatenate([[0], np.cumsum(T_b)])
    pl.T_tot = int(pl.off_b[-1])

    pl.cores = []
    for k in range(NCORES):
        s_k, d_k, kind = per_core[k]
        T_tot = pl.T_tot
        slot_src = np.zeros((128, T_tot), np.int64)     # default pad: row 0
        jv = np.full((128, T_tot), PAD_J, np.float32)
        bounds = np.searchsorted(d_k // BLK, np.arange(nblk + 1))
        for b in range(nblk):
            lo, hi = bounds[b], bounds[b + 1]
            cnt_k = hi - lo
            base_t = pl.off_b[b]
            for t in range((cnt_k + TILE - 1) // TILE):
                a = lo + t * TILE
                c = min(TILE, cnt_k - t * TILE)
                slot_src[0:c, base_t + t] = s_k[a:a + c]
                jv[0:c, base_t + t] = d_k[a:a + c] - b * BLK
        cp = _P()
        cp.slot_src, cp.jv = slot_src, jv
        pl.cores.append(cp)
    return pl


def _streams(pl, k):
    cp = pl.cores[k]
    T_tot, nblk = pl.T_tot, pl.nblk
    # Q7 idx stream, wrapped [16, NI/16] replicated to 8 groups; per-slot
    # flat order i = t*128 + p  (matches dma_gather out [128, t, :])
    flat = cp.slot_src.T.reshape(-1).astype(np.int64)      # [T_tot*128]
    sub = (flat // 8).astype(np.int16)
    idxw = np.tile(sub.reshape(-1, 16).T, (8, 1))          # [128, NI/16]
    par = np.stack([(cp.slot_src >> b) & 1 for b in range(3)],
                   axis=0).astype(np.float32)              # [3, 128, T_tot]
    # one-hot ST[d, t, p] = (j[p, t] == d)
    j = cp.jv                                              # [128(p), T_tot]
    ST = (np.arange(128, dtype=np.float32)[:, None, None] ==
          j.T[None, :, :]).astype(np.float32)              # [d, T_tot, p]
    return idxw, np.ascontiguousarray(ST), par


# ---------------------------------------------------------------- program

def build_program(pl, want_debug=False):
    import concourse.bass as bass
    import concourse.bacc as bacc
    import concourse.tile as tile
    import concourse.mybir as mybir

    F32 = mybir.dt.float32
    BF16 = mybir.dt.bfloat16
    I16 = mybir.dt.int16
    AF = mybir.ActivationFunctionType
    ALU = mybir.AluOpType

    n_nodes = pl.n_nodes
    ndst, nblk, npad = pl.ndst, pl.nblk, pl.npad
    T_b, off_b, T_tot = pl.T_b, pl.off_b, pl.T_tot
    lo_lim = pl.lo_lim
    hi_rows = n_nodes - lo_lim

    nc = bacc.Bacc("TRN2", target_bir_lowering=False, debug=want_debug,
                   num_devices=NCORES)
    # -------- inputs
    XET = nc.dram_tensor("XET", [128, T_tot * 128], BF16, kind="ExternalInput")
    xoT = nc.dram_tensor("xoT", [128, npad], BF16, kind="ExternalInput")
    W1e = nc.dram_tensor("W1e", [128, R1_W], BF16, kind="ExternalInput")
    Vd1 = nc.dram_tensor("Vd1", [128, HEADS], BF16, kind="ExternalInput")
    WV2 = nc.dram_tensor("WV2", [128, 2, 18], BF16, kind="ExternalInput")
    IDENT = nc.dram_tensor("IDENT", [128, 128], BF16, kind="ExternalInput")
    IOTA = nc.dram_tensor("IOTA", [128, 128], BF16, kind="ExternalInput")
    B1R = nc.dram_tensor("B1R", [128, F1], BF16, kind="ExternalInput")
    B2R = nc.dram_tensor("B2R", [128, OUT_CH], F32, kind="ExternalInput")
    RECB = nc.dram_tensor("RECB", [128, 18], F32, kind="ExternalInput")
    JB = nc.dram_tensor("JB", [128, T_tot], BF16, kind="ExternalInput")
    STT = nc.dram_tensor("STT", [128, T_tot, 128], BF16, kind="ExternalInput")
    IDXW = nc.dram_tensor("IDXW", [128, T_tot * 8], I16, kind="ExternalInput")
    PARB = nc.dram_tensor("PARB", [3, 128, T_tot], BF16,
                          kind="ExternalInput")
    OUT = nc.dram_tensor("OUT", [ndst, OUT_CH], F32, kind="ExternalOutput")

    with tile.TileContext(nc) as tc:
        with (
            tc.tile_pool(name="dram", bufs=1, space="DRAM") as dpool,
            tc.tile_pool(name="const", bufs=1) as cpool,
            tc.tile_pool(name="persist", bufs=1) as ppool,
            tc.tile_pool(name="edge", bufs=4) as epool,
            tc.tile_pool(name="zp", bufs=2) as zpool,
            tc.tile_pool(name="wt", bufs=3) as wpool,
            tc.tile_pool(name="sel", bufs=3) as selpool,
            tc.tile_pool(name="small", bufs=3) as spool,
            tc.tile_pool(name="ps_r", bufs=3, space="PSUM") as ps_r,
            tc.tile_pool(name="ps_e", bufs=1, space="PSUM") as ps_e,
            tc.tile_pool(name="ps_a", bufs=2, space="PSUM") as ps_a,
            tc.tile_pool(name="ps_m", bufs=1, space="PSUM") as ps_m,
        ):
            REC = dpool.tile([ndst, REC_W], BF16)
            R2 = dpool.tile([n_nodes, REC_W], BF16, addr_space="Shared")

            # consts
            cW1e = cpool.tile([128, R1_W], BF16)
            cVd1 = cpool.tile([128, HEADS], BF16)
            cWV2 = cpool.tile([128, 2, 18], BF16)
            cID = cpool.tile([128, 128], BF16)
            cIO = cpool.tile([128, 128], BF16)
            cB1 = cpool.tile([128, F1], BF16)
            cB2 = cpool.tile([128, OUT_CH], F32)
            cRB = cpool.tile([128, 18], F32)
            for t_, s_ in ((cW1e, W1e), (cVd1, Vd1), (cWV2, WV2),
                           (cID, IDENT), (cIO, IOTA), (cB1, B1R),
                           (cB2, B2R), (cRB, RECB)):
                nc.sync.dma_start(t_[:], s_[:])

            tJB = ppool.tile([128, T_tot], BF16)
            nc.sync.dma_start(tJB[:], JB[:])
            tIX = ppool.tile([128, T_tot * 8], I16)
            nc.sync.dma_start(tIX[:], IDXW[:])
            tPAR = ppool.tile([128, 3, T_tot], BF16)
            nc.sync.dma_start(tPAR[:],
                              PARB[:].rearrange("b p t -> p b t"))
            xo = ppool.tile([128, npad], BF16)
            nc.sync.dma_start(xo[:], xoT[:])
            alD2 = ppool.tile([128, nblk, 1], BF16)

            # ---------------- shared per-block attention + aggregation
            TMAX = int(T_b.max())

            def attn_block(b, src_of_tile, F, H, alD_rhs, pa):
                """src_of_tile(t) -> [128, F+H] bf16/psum record AP with
                cols F:F+H = al_src. Accumulates pa [128, F+H]."""
                T = int(T_b[b])
                off = int(off_b[b])
                st1 = selpool.tile([128, TMAX, 128], BF16, tag=f"st{F}")
                nc.sync.dma_start(st1[:, 0:T, :], STT[:, off:off + T, :])
                S = selpool.tile([128, TMAX, 128], BF16, tag=f"S{F}")
                nc.vector.tensor_tensor(
                    S[:, 0:T, :],
                    tJB[:, off:off + T].unsqueeze(2)
                    .broadcast_to([128, T, 128]),
                    cIO[:].unsqueeze(1).broadcast_to([128, T, 128]),
                    op=ALU.is_equal)
                # e_dst for all tiles -> psum [128, T*H]
                pe = ps_e.tile([128, TMAX * HEADS], F32, tag="pe",
                               padded_shape=[128, 512])
                for t in range(T):
                    nc.tensor.matmul(pe[:, t * H:(t + 1) * H],
                                     st1[:, t, :], alD_rhs,
                                     start=True, stop=True)
                peb = spool.tile([128, TMAX, H], BF16, tag=f"peb{F}")
                nc.scalar.copy(peb[:, 0:T, :],
                               pe[:, 0:T * H].rearrange("p (t h) -> p t h",
                                                        h=H))
                # stage records into wt block tile
                wt = wpool.tile([128, TMAX, F + H], BF16, tag=f"wt{F}")
                for t in range(T):
                    rsrc = src_of_tile(t)
                    nc.scalar.copy(wt[:, t, :], rsrc)
                ecols = wt[:, 0:T, F:F + H]
                nc.vector.tensor_add(ecols, ecols, peb[:, 0:T, :])
                nc.vector.scalar_tensor_tensor(
                    ecols, ecols, NEG_SLOPE, ecols,
                    op0=ALU.mult, op1=ALU.max)
                nc.scalar.activation(ecols, ecols, AF.Exp)
                C = F // H
                nc.vector.tensor_mul(
                    wt[:, 0:T, 0:F].rearrange("p t (h c) -> p t h c", c=C),
                    wt[:, 0:T, 0:F].rearrange("p t (h c) -> p t h c", c=C),
                    wt[:, 0:T, F:F + H].unsqueeze(3).broadcast_to(
                        [128, T, H, C]))
                for t in range(T):
                    nc.tensor.matmul(pa[:], S[:, t, :], wt[:, t, :],
                                     start=(t == 0), stop=(t == T - 1))

            # ---------------- E1
            for b in range(nblk):
                T = int(T_b[b])
                off = int(off_b[b])
                xet = epool.tile([128, TMAX * 128], BF16, tag="xet")
                nc.sync.dma_start(xet[:, 0:T * 128],
                                  XET[:, off * 128:(off + T) * 128])
                # al_dst for own block
                pd = ps_r.tile([128, HEADS], F32, tag="rec",
                               padded_shape=[128, 512])
                nc.tensor.matmul(pd[:], xo[:, b * 128:(b + 1) * 128],
                                 cVd1[:], start=True, stop=True)
                alD1b = spool.tile([128, HEADS], BF16, tag="alD1b")
                nc.scalar.copy(alD1b[:], pd[:])

                pa = ps_a.tile([128, R1_W], F32, tag="pa",
                               padded_shape=[128, 512])

                def src1(t):
                    p = ps_r.tile([128, R1_W], F32, tag="rec",
                                  padded_shape=[128, 512])
                    nc.tensor.matmul(p[:], xet[:, t * 128:(t + 1) * 128],
                                     cW1e[:], start=True, stop=True)
                    return p[:]

                attn_block(b, src1, F1, HEADS, alD1b[:], pa)

                # finalize layer 1
                den = spool.tile([128, HEADS], F32, tag="den")
                nc.scalar.activation(den[:], pa[:, F1:F1 + HEADS], AF.Copy,
                                     bias=DEN_EPS)
                rden = spool.tile([128, HEADS], F32, tag="rden")
                nc.vector.reciprocal(rden[:], den[:])
                h2t = spool.tile([128, F1], BF16, tag="h2t")
                nc.vector.tensor_mul(
                    h2t[:].rearrange("p (h c) -> p h c", c=HID),
                    pa[:, 0:F1].rearrange("p (h c) -> p h c", c=HID),
                    rden[:].unsqueeze(2).broadcast_to([128, HEADS, HID]))
                nc.vector.tensor_add(h2t[:], h2t[:], cB1[:])
                t1 = spool.tile([128, F1], BF16, tag="t1")
                nc.vector.tensor_scalar_min(t1[:], h2t[:], 0.0)
                nc.scalar.activation(t1[:], t1[:], AF.Exp)
                nc.vector.tensor_scalar_max(h2t[:], h2t[:], 0.0)
                nc.vector.tensor_add(h2t[:], h2t[:], t1[:])
                ptr = ps_m.tile([128, 2, 128], BF16, tag="m",
                                padded_shape=[128, 2, 256])
                nc.tensor.transpose(ptr[:, 0, :], h2t[:, 0:128], cID[:])
                nc.tensor.transpose(ptr[:, 1, :], h2t[:, 128:256], cID[:])
                h2T = spool.tile([128, 2, 128], BF16, tag="h2T")
                nc.scalar.copy(h2T[:], ptr[:])
                prc = ps_m.tile([128, 18], F32, tag="m2",
                                padded_shape=[128, 512])
                nc.tensor.matmul(prc[:], h2T[:, 0, :], cWV2[:, 0, :],
                                 start=True, stop=False)
                nc.tensor.matmul(prc[:], h2T[:, 1, :], cWV2[:, 1, :],
                                 start=False, stop=True)
                rec = spool.tile([128, REC_W], BF16, tag="rec18")
                nc.vector.memset(rec[:, 18:REC_W], 0.0)
                nc.vector.tensor_add(rec[:, 0:18], prc[:], cRB[:])
                nc.scalar.copy(alD2[:, b, :], rec[:, 17:18])
                m = min(BLK, ndst - b * BLK)
                nc.sync.dma_start(REC[b * BLK:b * BLK + m, :], rec[0:m, :])

            # ---------------- AllGather
            nc.gpsimd.collective_compute(
                "AllGather", mybir.AluOpType.bypass,
                replica_groups=[list(range(NCORES))],
                ins=[REC.opt()], outs=[R2.opt()])

            # ---------------- E2 (8-packed gather: 512B reads cover 8
            # records; idx = src//8 fits int16; 3-stage bit-select on DVE)
            tblp = R2[:, :].rearrange("(a b) w -> a (b w)", b=8)
            for b in range(nblk):
                T = int(T_b[b])
                off = int(off_b[b])
                z = zpool.tile([128, TMAX, 8 * REC_W], BF16, tag="z")
                GCAP = 8
                for t0 in range(0, T, GCAP):
                    t1_ = min(t0 + GCAP, T)
                    ni = (t1_ - t0) * TILE
                    nc.gpsimd.dma_gather(
                        z[:, t0:t1_, :], tblp,
                        tIX[:, (off + t0) * 8:(off + t1_) * 8],
                        ni, ni, 8 * REC_W)

                def bitsel(dst, nk, w, even, odd, bit):
                    par = tPAR[:, bit, off:off + T].unsqueeze(2)                         .unsqueeze(3).broadcast_to([128, T, nk, w])
                    nc.vector.tensor_sub(dst, odd, even)
                    nc.vector.tensor_mul(dst, dst, par)
                    nc.vector.tensor_add(dst, dst, even)

                zv = z[:, 0:T, :].rearrange("p t (a b w) -> p t a b w",
                                            b=2, w=REC_W)
                zA = zpool.tile([128, TMAX, 4, REC_W2], BF16, tag="zA")
                bitsel(zA[:, 0:T, :, :], 4, REC_W2,
                       zv[:, :, :, 0, 0:REC_W2], zv[:, :, :, 1, 0:REC_W2], 0)
                zAv = zA[:, 0:T, :, :].rearrange("p t (a b) w -> p t a b w",
                                                 b=2)
                zB = zpool.tile([128, TMAX, 2, REC_W2], BF16, tag="zB")
                bitsel(zB[:, 0:T, :, :], 2, REC_W2,
                       zAv[:, :, :, 0, :], zAv[:, :, :, 1, :], 1)
                zBv = zB[:, 0:T, :, :].rearrange("p t (a b) w -> p t a b w",
                                                 b=2)
                z17 = zpool.tile([128, TMAX, 1, REC_W2], BF16, tag="z17")
                bitsel(z17[:, 0:T, :, :], 1, REC_W2,
                       zBv[:, :, :, 0, :], zBv[:, :, :, 1, :], 2)
                # transpose alD2 column to row, replicate not needed:
                # e_d2 via ST-expansion with rhs alD2 block column
                pa2 = ps_a.tile([128, OUT_CH + 1], F32, tag="pa",
                                padded_shape=[128, 512])

                def src2(t):
                    return z17[:, t, 0, 0:OUT_CH + 1]

                attn_block(b, src2, OUT_CH, 1, alD2[:, b, :], pa2)

                den2 = spool.tile([128, 1], F32, tag="den2")
                nc.scalar.activation(den2[:], pa2[:, OUT_CH:OUT_CH + 1],
                                     AF.Copy, bias=DEN_EPS)
                rden2 = spool.tile([128, 1], F32, tag="rden2")
                nc.vector.reciprocal(rden2[:], den2[:])
                v = spool.tile([128, OUT_CH], F32, tag="v")
                nc.vector.tensor_mul(
                    v[:], pa2[:, 0:OUT_CH],
                    rden2[:].broadcast_to([128, OUT_CH]))
                nc.vector.tensor_add(v[:], v[:], cB2[:])
                mx = spool.tile([128, 1], F32, tag="mx")
                nc.vector.tensor_reduce(mx[:], v[:], op=ALU.max,
                                        axis=mybir.AxisListType.X)
                nc.vector.tensor_sub(
                    v[:], v[:], mx[:].broadcast_to([128, OUT_CH]))
                ex = spool.tile([128, OUT_CH], F32, tag="exf")
                sm = spool.tile([128, 1], F32, tag="sm")
                nc.scalar.activation(ex[:], v[:], AF.Exp, accum_out=sm[:])
                lns = spool.tile([128, 1], F32, tag="lns")
                nc.scalar.activation(lns[:], sm[:], AF.Ln)
                nc.vector.tensor_sub(
                    v[:], v[:], lns[:].broadcast_to([128, OUT_CH]))
                m = min(BLK, ndst - b * BLK)
                nc.sync.dma_start(OUT[b * BLK:b * BLK + m, :], v[0:m, :])

    nc.compile()
    return nc


# ---------------------------------------------------------------- host prep

def _bf16(a):
    import ml_dtypes
    return np.asarray(a, np.float32).astype(ml_dtypes.bfloat16)


def _host_inputs(pl, inputs):
    x = np.ascontiguousarray(np.asarray(inputs["x"], np.float32))
    W1 = np.asarray(inputs["W1"], np.float32)
    a_s1 = np.asarray(inputs["a_src1"], np.float32)
    a_d1 = np.asarray(inputs["a_dst1"], np.float32)
    b1 = np.asarray(inputs["b1"], np.float32)
    W2 = np.asarray(inputs["W2"], np.float32)
    a_s2 = np.asarray(inputs["a_src2"], np.float32)
    a_d2 = np.asarray(inputs["a_dst2"], np.float32)
    b2 = np.asarray(inputs["b2"], np.float32)
    n_nodes, ndst, nblk, npad = pl.n_nodes, pl.ndst, pl.nblk, pl.npad

    A_s1 = np.zeros((F1, HEADS), np.float32)
    A_d1 = np.zeros((F1, HEADS), np.float32)
    for h in range(HEADS):
        A_s1[h * HID:(h + 1) * HID, h] = a_s1[h]
        A_d1[h * HID:(h + 1) * HID, h] = a_d1[h]
    V_s1 = (W1 @ A_s1).astype(np.float32)
    V_d1 = (W1 @ A_d1).astype(np.float32)
    V_s2 = (W2 @ a_s2[0]).astype(np.float32)
    V_d2 = (W2 @ a_d2[0]).astype(np.float32)
    WV2 = np.concatenate([W2, V_s2[:, None], V_d2[:, None]], axis=1)
    RECB = -WV2.sum(axis=0, keepdims=True)

    iota = np.tile(np.arange(128, dtype=np.float32)[None, :], (128, 1))
    xbf = _bf16(x)
    common = {
        "W1e": _bf16(np.concatenate([W1, V_s1], axis=1)),
        "Vd1": _bf16(V_d1),
        "WV2": _bf16(WV2.reshape(2, 128, 18).transpose(1, 0, 2)),
        "IDENT": _bf16(np.eye(128, dtype=np.float32)),
        "IOTA": _bf16(iota),
        "B1R": _bf16(np.tile(b1[None, :], (128, 1))),
        "B2R": np.tile(b2[None, :], (128, 1)).astype(np.float32),
        "RECB": np.tile(RECB, (128, 1)).astype(np.float32),
    }
    in_maps = []
    for k in range(NCORES):
        cp = pl.cores[k]
        idxw, ST, par = _streams(pl, k)
        # x_eT: [128 feats, T_tot*128], column slot (t, p) = x[src]
        xe = xbf[cp.slot_src.T.reshape(-1)]       # [T_tot*128, 128]
        xo = np.zeros((npad, 128), np.float32)
        xo[:ndst] = x[k * ndst:(k + 1) * ndst]
        m = dict(common)
        m["XET"] = np.ascontiguousarray(xe.T)
        m["xoT"] = _bf16(xo.T)
        m["JB"] = _bf16(cp.jv)
        m["STT"] = _bf16(ST)
        m["PARB"] = _bf16(par)
        m["IDXW"] = np.ascontiguousarray(idxw)
        in_maps.append(m)
    return in_maps


# ---------------------------------------------------------------- entry

def _run(inputs, trace=False, **kw):
    from concourse.bass_utils import run_bass_kernel_spmd

    edge_index = np.asarray(inputs["edge_index"])
    n_nodes = int(np.asarray(inputs["x"]).shape[0])
    pl = _plan(edge_index, n_nodes)
    nc = build_program(pl)
    in_maps = _host_inputs(pl, inputs)
    res = run_bass_kernel_spmd(nc, in_maps, list(range(NCORES)),
                               trace=trace, **kw)
    out = np.concatenate([res.results[k]["OUT"] for k in range(NCORES)],
                         axis=0)
    return out.astype(np.float32), res


def kernel(**inputs):
    out, _ = _run(inputs)
    return out

